# revision 26
# baseline (speedup 1.0000x reference)
"""ChannelAttention Trainium2 Bass kernel (v2: bf16 matmul path, CHUNK=512).

Problem: B=16, N=4096, C=768, H=8, d=96
  qkv = x @ w_qkv; q,k,v per head [d, N]; q,k l2-normalized over N;
  attn = softmax(q_hat @ k_hat^T * temp); out = attn @ v; y = out @ w_proj + b_proj

Distribution: data-parallel over B across 8 cores (2 batches/core). No collectives.

Per-core dataflow (per batch, N split into 8 chunks of 512 tokens):
  Phase A (per chunk):
    - DMA x chunk [512, 768] f32 -> PE-transpose (f32) -> xT bf16 [768, 512]
    - qk_nat [512, 1536] = xT.T @ w_qkv[:, :1536]  (bf16, PSUM f32 -> bf16 SBUF)
    - sq = qk^2 (ACT Square, bf16); norm row-partials = ones.T @ sq (PE matvec)
      accumulated into norm_acc [1, 1536]
    - Gram_h += q_h.T @ k_h accumulated in PSUM over all chunks (bf16)
    - vT_h [96, 512] = w_v_h.T @ xT (bf16) -> DRAM scratch
  Phase B (per batch, vectorized over heads):
    rnorm = 1/sqrt(norm_acc); rq *= temp; S_h = rq_h (x) rk_h (rank-1 PE);
    att = Gram * S; softmax via DVE max/sub + ACT Exp + DVE sum/recip;
    attnT_h = att_h.T (PE, bf16)
  Phase C (per chunk): av_h = attnT_h.T @ vT_h; out_h = av * rsum (ACT scale);
    y = sum_h out_h.T @ w_proj_h; y += b_proj (DVE, partition-broadcast bias);
"""

import numpy as np

B, N, C = 16, 4096, 768
H, D = 8, 96
N_CORES = 8
B_LOC = B // N_CORES          # 2 batches per core
CHUNK = 512                   # tokens per chunk
NCH = N // CHUNK              # 8 chunks
SUB = CHUNK // 128            # 4 token-subtiles per chunk
KT = C // 128                 # 6 contraction subtiles
FC_QK = 512                   # qk free chunk
FC_Y = 384                    # proj free chunk (2 x 384 = 768)

_CACHE = {}


def _build():
    if "nc" in _CACHE:
        return _CACHE["nc"]

    import concourse.bass as bass
    import concourse.mybir as mybir
    import concourse.tile as tile
    from concourse import bacc
    from concourse.masks import make_identity

    f32 = mybir.dt.float32
    bf16 = mybir.dt.bfloat16
    AF = mybir.ActivationFunctionType

    nc = bacc.Bacc("TRN2", target_bir_lowering=False, debug=False)

    x_d = nc.dram_tensor("x", [B_LOC, N, C], f32, kind="ExternalInput").ap()
    wqkv_d = nc.dram_tensor("w_qkv", [C, 3 * C], f32, kind="ExternalInput").ap()
    temp_d = nc.dram_tensor("temperature", [H, 1, 1], f32, kind="ExternalInput").ap()
    wproj_d = nc.dram_tensor("w_proj", [C, C], f32, kind="ExternalInput").ap()
    bproj_d = nc.dram_tensor("b_proj", [C], f32, kind="ExternalInput").ap()
    y_d = nc.dram_tensor("y", [B_LOC, N, C], f32, kind="ExternalOutput").ap()

    with tile.TileContext(nc) as tc:
        import contextlib
        with contextlib.ExitStack() as ctx:
            singles = ctx.enter_context(tc.tile_pool(name="singles", bufs=1))
            perb = ctx.enter_context(tc.tile_pool(name="perb", bufs=1))
            rot = ctx.enter_context(tc.tile_pool(name="rot", bufs=2))
            rot1 = ctx.enter_context(tc.tile_pool(name="rot1", bufs=2))
            ps = ctx.enter_context(tc.tile_pool(name="ps", bufs=2, space="PSUM"))
            ps3 = ctx.enter_context(tc.tile_pool(name="ps3", bufs=2, space="PSUM"))
            dram = ctx.enter_context(tc.tile_pool(name="dram", bufs=1, space="DRAM"))

            # ---- constants / weights (resident) ----
            ident = singles.tile([128, 128], f32, tag="ident")
            make_identity(nc, ident)
            identb = singles.tile([128, 128], bf16, tag="identb")
            nc.vector.tensor_copy(identb[:], ident[:])
            ones_bf = singles.tile([128, 1], bf16, tag="ones_bf")
            nc.vector.memset(ones_bf[:], 1.0)
            eps_sb = singles.tile([1, 1], f32, tag="eps")
            nc.vector.memset(eps_sb[:], 1e-24)

            # prefetch first x chunk before the big weight DMAs share the queue
            x_nat0 = rot.tile([128, SUB, C], bf16, tag="x_nat", name="x_nat0")
            nc.gpsimd.dma_start(
                x_nat0[:],
                x_d[0, 0:CHUNK, :].rearrange("(s p) c -> p s c", p=128))
            x_nat1 = rot.tile([128, SUB, C], bf16, tag="x_nat", name="x_nat1")
            nc.gpsimd.dma_start(
                x_nat1[:],
                x_d[0, CHUNK:2 * CHUNK, :].rearrange("(s p) c -> p s c", p=128))

            # bf16 weights via casting gpsimd DMA; qk weights in 3 pieces so
            # the first matmul group waits on only 1.6MB
            wqk0 = singles.tile([128, KT, FC_QK], bf16, tag="wqk0")
            wqk1 = singles.tile([128, KT, FC_QK], bf16, tag="wqk1")
            wqk2 = singles.tile([128, KT, FC_QK], bf16, tag="wqk2")
            wqk_t = [wqk0, wqk1, wqk2]
            for fc in range(3):
                nc.gpsimd.dma_start(
                    wqk_t[fc][:],
                    wqkv_d[:, fc * FC_QK:(fc + 1) * FC_QK]
                    .rearrange("(ko p) n -> p ko n", p=128))
            wv_sb = singles.tile([128, KT, C], bf16, tag="wv")
            nc.gpsimd.dma_start(
                wv_sb[:],
                wqkv_d[:, 2 * C:3 * C].rearrange("(ko p) n -> p ko n", p=128))
            wp_sb = singles.tile([D, H, C], bf16, tag="wproj")
            nc.gpsimd.dma_start(
                wp_sb[:], wproj_d.rearrange("(h d) n -> d h n", h=H))
            # bias broadcast to all 128 partitions (f32, added on DVE)
            b128 = singles.tile([128, C], f32, tag="b128")
            nc.gpsimd.dma_start(
                b128[:],
                bass.AP(tensor=bproj_d.tensor, offset=0,
                        ap=[[0, 128], [1, C]]))
            temp_sb = singles.tile([1, H], f32, tag="temp")
            nc.sync.dma_start(temp_sb[:], temp_d.rearrange("h x y -> (x y) h"))

            for b in range(B_LOC):
                # ---- per-batch persistent ----
                vt_dram = dram.tile([D, H, N], bf16, tag="vt_dram")
                gram_sb = perb.tile([D, H, D], f32, tag="gram_sb")
                nc.vector.memset(gram_sb[:], 0.0)
                norm_acc = perb.tile([1, 2 * C], f32, tag="norm_acc")
                nc.vector.memset(norm_acc[:], 0.0)
                attnT = perb.tile([D, H, D], bf16, tag="attnT")
                rsums = perb.tile([D, H], f32, tag="rsums")

                # ================= Phase A =================
                for ci in range(NCH):
                    c0 = ci * CHUNK
                    if b == 0 and ci == 0:
                        x_nat = x_nat0
                    elif b == 0 and ci == 1:
                        x_nat = x_nat1
                    else:
                        x_nat = rot.tile([128, SUB, C], bf16, tag="x_nat")
                        nc.gpsimd.dma_start(
                            x_nat[:],
                            x_d[b, c0:c0 + CHUNK, :]
                            .rearrange("(s p) c -> p s c", p=128))

                    # transpose x chunk -> xT bf16 [c-part, kt, n]
                    xT = rot.tile([128, KT, CHUNK], bf16, tag="xT")
                    for grp in range(SUB * KT // 4):  # 24 transposes, 4/psum tile
                        ptr = ps.tile([128, 4, 128], bf16, tag="tr")
                        for j in range(4):
                            blk = grp * 4 + j
                            s, cb = blk // KT, blk % KT
                            nc.tensor.transpose(
                                ptr[:, j, :], x_nat[:, s, cb * 128:(cb + 1) * 128],
                                identb[:])
                        for j in range(4):
                            blk = grp * 4 + j
                            s, cb = blk // KT, blk % KT
                            nc.vector.tensor_copy(
                                xT[:, cb, s * 128:(s + 1) * 128], ptr[:, j, :])

                    # qk_nat (natural layout) + squares + norm partials
                    qk = rot.tile([128, SUB, 2 * C], bf16, tag="qk")
                    sq = rot1.tile([128, SUB, 2 * C], bf16, tag="sq")
                    for s in range(SUB):
                        for fc in range(3):
                            pqk = ps3.tile([128, FC_QK], f32, tag="qkmv")
                            for k in range(KT):
                                nc.tensor.matmul(
                                    pqk[:],
                                    xT[:, k, s * 128:(s + 1) * 128],
                                    wqk_t[fc][:, k, :],
                                    start=(k == 0), stop=(k == KT - 1))
                            nc.vector.tensor_copy(
                                qk[:, s, fc * FC_QK:(fc + 1) * FC_QK], pqk[:])
                            nc.scalar.activation(
                                sq[:, s, fc * FC_QK:(fc + 1) * FC_QK], pqk[:],
                                AF.Square)
                    # norm partials: [1, 512] = ones.T @ sq, accum over subtiles
                    for fc in range(3):
                        pmv = ps3.tile([1, FC_QK], f32, tag="qkmv")
                        for s in range(SUB):
                            nc.tensor.matmul(
                                pmv[:], ones_bf[:],
                                sq[:, s, fc * FC_QK:(fc + 1) * FC_QK],
                                start=(s == 0), stop=(s == SUB - 1))
                        nc.vector.tensor_add(
                            norm_acc[:, fc * FC_QK:(fc + 1) * FC_QK],
                            norm_acc[:, fc * FC_QK:(fc + 1) * FC_QK], pmv[:])

                    # Gram partial per head (bf16) -> accumulate in SBUF
                    for h in range(H):
                        pg = ps.tile([D, D], f32, tag="gr")
                        for s in range(SUB):
                            nc.tensor.matmul(
                                pg[:],
                                qk[:, s, h * D:(h + 1) * D],
                                qk[:, s, C + h * D:C + (h + 1) * D],
                                start=(s == 0), stop=(s == SUB - 1))
                        nc.vector.tensor_add(gram_sb[:, h, :], gram_sb[:, h, :],
                                             pg[:])

                    # vT per head (bf16) -> DRAM scratch
                    vtc = rot.tile([D, H, CHUNK], bf16, tag="vtc")
                    for h in range(H):
                        pvt = ps3.tile([D, CHUNK], f32, tag="vt")
                        for k in range(KT):
                            nc.tensor.matmul(
                                pvt[:],
                                wv_sb[:, k, h * D:(h + 1) * D],
                                xT[:, k, :],
                                start=(k == 0), stop=(k == KT - 1))
                        nc.scalar.copy(vtc[:, h, :], pvt[:])
                    nc.sync.dma_start(vt_dram[:, :, c0:c0 + CHUNK], vtc[:])

                # ================= Phase B (vectorized over heads) ===========
                rn = perb.tile([1, 2 * C], f32, tag="rn")
                nc.scalar.activation(rn[:], norm_acc[:], AF.Abs_reciprocal_sqrt,
                                     bias=eps_sb[:])
                nc.vector.tensor_tensor(
                    rn[0:1, 0:C].rearrange("o (h d) -> o h d", h=H),
                    rn[0:1, 0:C].rearrange("o (h d) -> o h d", h=H),
                    temp_sb[:].unsqueeze(2).to_broadcast([1, H, D]),
                    mybir.AluOpType.mult)

                # S = rq (x) rk packed like gram: bank0 heads 0-4, bank1 heads 5-7
                S_ps0 = ps3.tile([128, 480], f32, tag="qkmv", name="S_ps0")
                S_ps1 = ps3.tile([128, 288], f32, tag="qkmv", name="S_ps1")
                S_ps = [S_ps0, S_ps1]
                for h in range(H):
                    bank, off = (0, h * D) if h < 5 else (1, (h - 5) * D)
                    nc.tensor.matmul(
                        S_ps[bank][0:D, off:off + D],
                        rn[0:1, h * D:(h + 1) * D],
                        rn[0:1, C + h * D:C + (h + 1) * D],
                        start=True, stop=True)
                att = perb.tile([D, H, D], f32, tag="att")
                nc.vector.tensor_tensor(
                    att[:, 0:5, :].rearrange("p h d -> p (h d)"),
                    gram_sb[:, 0:5, :].rearrange("p h d -> p (h d)"),
                    S_ps[0][0:D, :], mybir.AluOpType.mult)
                nc.vector.tensor_tensor(
                    att[:, 5:8, :].rearrange("p h d -> p (h d)"),
                    gram_sb[:, 5:8, :].rearrange("p h d -> p (h d)"),
                    S_ps[1][0:D, :], mybir.AluOpType.mult)
                # softmax over last axis, all heads at once
                maxs = perb.tile([D, H], f32, tag="maxs")
                nc.vector.tensor_reduce(
                    maxs[:], att[:], axis=mybir.AxisListType.X,
                    op=mybir.AluOpType.max)
                nc.vector.tensor_tensor(
                    att[:], att[:],
                    maxs[:].unsqueeze(2).to_broadcast([D, H, D]),
                    mybir.AluOpType.subtract)
                atte = perb.tile([D, H, D], bf16, tag="atte")
                nc.scalar.activation(atte[:], att[:], AF.Exp)
                ssum = perb.tile([D, H], f32, tag="ssum")
                nc.vector.tensor_reduce(
                    ssum[:], atte[:], axis=mybir.AxisListType.X,
                    op=mybir.AluOpType.add)
                nc.vector.reciprocal(rsums[:], ssum[:])
                # attnT_h = atte_h.T (bf16), packed 4 per psum tile
                for grp in range(2):
                    pT = ps3.tile([D, 4, D], bf16, tag="vt")
                    for j in range(4):
                        h = grp * 4 + j
                        nc.tensor.transpose(
                            pT[:, j, :], atte[:, h, :], identb[0:D, 0:D])
                    nc.vector.tensor_copy(
                        attnT[:, grp * 4:(grp + 1) * 4, :], pT[:])

                # ================= Phase C =================
                for ci in range(NCH):
                    c0 = ci * CHUNK
                    vtc2 = rot.tile([D, H, CHUNK], bf16, tag="vtc2")
                    nc.sync.dma_start(vtc2[:], vt_dram[:, :, c0:c0 + CHUNK])
                    outh = rot.tile([D, H, CHUNK], bf16, tag="outh")
                    for h in range(H):
                        pav = ps3.tile([D, CHUNK], f32, tag="vt")
                        nc.tensor.matmul(pav[:], attnT[:, h, :], vtc2[:, h, :],
                                         start=True, stop=True)
                        nc.scalar.activation(outh[:, h, :], pav[:], AF.Copy,
                                             scale=rsums[:, h:h + 1])
                    for s in range(SUB):
                        y_sb = rot.tile([128, C], f32, tag="y_sb")
                        for fc in range(2):
                            pY = ps3.tile([128, FC_Y], f32, tag="qkmv")
                            for h in range(H):
                                nc.tensor.matmul(
                                    pY[:],
                                    outh[:, h, s * 128:(s + 1) * 128],
                                    wp_sb[:, h, fc * FC_Y:(fc + 1) * FC_Y],
                                    start=(h == 0), stop=(h == H - 1))
                            nc.vector.tensor_tensor(
                                y_sb[:, fc * FC_Y:(fc + 1) * FC_Y], pY[:],
                                b128[:, fc * FC_Y:(fc + 1) * FC_Y],
                                mybir.AluOpType.add)
                        nc.sync.dma_start(
                            y_d[b, c0 + s * 128:c0 + (s + 1) * 128, :], y_sb[:])

    nc.compile()
    _CACHE["nc"] = nc
    return nc


def _run(inputs, trace=False):
    from concourse.bass_utils import run_bass_kernel_spmd

    nc = _build()
    x = np.ascontiguousarray(np.asarray(inputs["x"], dtype=np.float32))
    shards = x.reshape(N_CORES, B_LOC, N, C)
    common = {
        "w_qkv": np.asarray(inputs["w_qkv"], dtype=np.float32),
        "temperature": np.asarray(inputs["temperature"], dtype=np.float32),
        "w_proj": np.asarray(inputs["w_proj"], dtype=np.float32),
        "b_proj": np.asarray(inputs["b_proj"], dtype=np.float32),
    }
    in_maps = [{"x": shards[i], **common} for i in range(N_CORES)]
    res = run_bass_kernel_spmd(nc, in_maps, list(range(N_CORES)), trace=trace)
    out = np.concatenate([res.results[i]["y"] for i in range(N_CORES)], axis=0)
    return out.reshape(B, N, C), res


def kernel(**inputs) -> np.ndarray:
    out, _ = _run(inputs, trace=False)
    return out


def kernel_traced(**inputs):
    """Returns (output, BassKernelResults with exec_time_ns). Requires NTFF hook."""
    out, res = _run(inputs, trace=True)
    return out, res


# revision 27
# speedup vs baseline: 1.0021x; 1.0021x over previous
"""ChannelAttention Trainium2 Bass kernel (v2: bf16 matmul path, CHUNK=512).

Problem: B=16, N=4096, C=768, H=8, d=96
  qkv = x @ w_qkv; q,k,v per head [d, N]; q,k l2-normalized over N;
  attn = softmax(q_hat @ k_hat^T * temp); out = attn @ v; y = out @ w_proj + b_proj

Distribution: data-parallel over B across 8 cores (2 batches/core). No collectives.

Per-core dataflow (per batch, N split into 8 chunks of 512 tokens):
  Phase A (per chunk):
    - DMA x chunk [512, 768] f32 -> PE-transpose (f32) -> xT bf16 [768, 512]
    - qk_nat [512, 1536] = xT.T @ w_qkv[:, :1536]  (bf16, PSUM f32 -> bf16 SBUF)
    - sq = qk^2 (ACT Square, bf16); norm row-partials = ones.T @ sq (PE matvec)
      accumulated into norm_acc [1, 1536]
    - Gram_h += q_h.T @ k_h accumulated in PSUM over all chunks (bf16)
    - vT_h [96, 512] = w_v_h.T @ xT (bf16) -> DRAM scratch
  Phase B (per batch, vectorized over heads):
    rnorm = 1/sqrt(norm_acc); rq *= temp; S_h = rq_h (x) rk_h (rank-1 PE);
    att = Gram * S; softmax via DVE max/sub + ACT Exp + DVE sum/recip;
    attnT_h = att_h.T (PE, bf16)
  Phase C (per chunk): av_h = attnT_h.T @ vT_h; out_h = av * rsum (ACT scale);
    y = sum_h out_h.T @ w_proj_h; y += b_proj (DVE, partition-broadcast bias);
"""

import numpy as np

B, N, C = 16, 4096, 768
H, D = 8, 96
N_CORES = 8
B_LOC = B // N_CORES          # 2 batches per core
CHUNK = 512                   # tokens per chunk
NCH = N // CHUNK              # 8 chunks
SUB = CHUNK // 128            # 4 token-subtiles per chunk
KT = C // 128                 # 6 contraction subtiles
FC_QK = 512                   # qk free chunk
FC_Y = 384                    # proj free chunk (2 x 384 = 768)

_CACHE = {}


def _build():
    if "nc" in _CACHE:
        return _CACHE["nc"]

    import concourse.bass as bass
    import concourse.mybir as mybir
    import concourse.tile as tile
    from concourse import bacc
    from concourse.masks import make_identity

    f32 = mybir.dt.float32
    bf16 = mybir.dt.bfloat16
    AF = mybir.ActivationFunctionType

    nc = bacc.Bacc("TRN2", target_bir_lowering=False, debug=False)

    x_d = nc.dram_tensor("x", [B_LOC, N, C], f32, kind="ExternalInput").ap()
    wqkv_d = nc.dram_tensor("w_qkv", [C, 3 * C], f32, kind="ExternalInput").ap()
    temp_d = nc.dram_tensor("temperature", [H, 1, 1], f32, kind="ExternalInput").ap()
    wproj_d = nc.dram_tensor("w_proj", [C, C], f32, kind="ExternalInput").ap()
    bproj_d = nc.dram_tensor("b_proj", [C], f32, kind="ExternalInput").ap()
    y_d = nc.dram_tensor("y", [B_LOC, N, C], f32, kind="ExternalOutput").ap()

    with tile.TileContext(nc) as tc:
        import contextlib
        with contextlib.ExitStack() as ctx:
            singles = ctx.enter_context(tc.tile_pool(name="singles", bufs=1))
            perb = ctx.enter_context(tc.tile_pool(name="perb", bufs=1))
            rot = ctx.enter_context(tc.tile_pool(name="rot", bufs=2))
            rot1 = ctx.enter_context(tc.tile_pool(name="rot1", bufs=2))
            ps = ctx.enter_context(tc.tile_pool(name="ps", bufs=2, space="PSUM"))
            ps3 = ctx.enter_context(tc.tile_pool(name="ps3", bufs=2, space="PSUM"))
            dram = ctx.enter_context(tc.tile_pool(name="dram", bufs=1, space="DRAM"))

            # ---- constants / weights (resident) ----
            ident = singles.tile([128, 128], f32, tag="ident")
            make_identity(nc, ident)
            identb = singles.tile([128, 128], bf16, tag="identb")
            nc.vector.tensor_copy(identb[:], ident[:])
            ones_bf = singles.tile([128, 1], bf16, tag="ones_bf")
            nc.vector.memset(ones_bf[:], 1.0)
            eps_sb = singles.tile([1, 1], f32, tag="eps")
            nc.vector.memset(eps_sb[:], 1e-24)

            # startup DMA order on the single gpsimd cast queue: x0 first
            # (feeds the transposes), then the first qk-weight piece, then x1,
            # then the rest — so the first matmul group unblocks asap
            x_nat0 = rot.tile([128, SUB, C], bf16, tag="x_nat", name="x_nat0")
            nc.gpsimd.dma_start(
                x_nat0[:],
                x_d[0, 0:CHUNK, :].rearrange("(s p) c -> p s c", p=128))
            wqk0 = singles.tile([128, KT, FC_QK], bf16, tag="wqk0")
            wqk1 = singles.tile([128, KT, FC_QK], bf16, tag="wqk1")
            wqk2 = singles.tile([128, KT, FC_QK], bf16, tag="wqk2")
            wqk_t = [wqk0, wqk1, wqk2]
            nc.gpsimd.dma_start(
                wqk_t[0][:],
                wqkv_d[:, 0:FC_QK].rearrange("(ko p) n -> p ko n", p=128))
            x_nat1 = rot.tile([128, SUB, C], bf16, tag="x_nat", name="x_nat1")
            nc.gpsimd.dma_start(
                x_nat1[:],
                x_d[0, CHUNK:2 * CHUNK, :].rearrange("(s p) c -> p s c", p=128))
            for fc in range(1, 3):
                nc.gpsimd.dma_start(
                    wqk_t[fc][:],
                    wqkv_d[:, fc * FC_QK:(fc + 1) * FC_QK]
                    .rearrange("(ko p) n -> p ko n", p=128))
            wv_sb = singles.tile([128, KT, C], bf16, tag="wv")
            nc.gpsimd.dma_start(
                wv_sb[:],
                wqkv_d[:, 2 * C:3 * C].rearrange("(ko p) n -> p ko n", p=128))
            wp_sb = singles.tile([D, H, C], bf16, tag="wproj")
            nc.gpsimd.dma_start(
                wp_sb[:], wproj_d.rearrange("(h d) n -> d h n", h=H))
            # bias broadcast to all 128 partitions (f32, added on DVE)
            b128 = singles.tile([128, C], f32, tag="b128")
            nc.gpsimd.dma_start(
                b128[:],
                bass.AP(tensor=bproj_d.tensor, offset=0,
                        ap=[[0, 128], [1, C]]))
            temp_sb = singles.tile([1, H], f32, tag="temp")
            nc.sync.dma_start(temp_sb[:], temp_d.rearrange("h x y -> (x y) h"))

            for b in range(B_LOC):
                # ---- per-batch persistent ----
                vt_dram = dram.tile([D, H, N], bf16, tag="vt_dram")
                gram_sb = perb.tile([D, H, D], f32, tag="gram_sb")
                nc.vector.memset(gram_sb[:], 0.0)
                norm_acc = perb.tile([1, 2 * C], f32, tag="norm_acc")
                nc.vector.memset(norm_acc[:], 0.0)
                attnT = perb.tile([D, H, D], bf16, tag="attnT")
                rsums = perb.tile([D, H], f32, tag="rsums")

                # ================= Phase A =================
                for ci in range(NCH):
                    c0 = ci * CHUNK
                    if b == 0 and ci == 0:
                        x_nat = x_nat0
                    elif b == 0 and ci == 1:
                        x_nat = x_nat1
                    else:
                        x_nat = rot.tile([128, SUB, C], bf16, tag="x_nat")
                        nc.gpsimd.dma_start(
                            x_nat[:],
                            x_d[b, c0:c0 + CHUNK, :]
                            .rearrange("(s p) c -> p s c", p=128))

                    # transpose x chunk -> xT bf16 [c-part, kt, n]
                    xT = rot.tile([128, KT, CHUNK], bf16, tag="xT")
                    for grp in range(SUB * KT // 4):  # 24 transposes, 4/psum tile
                        ptr = ps.tile([128, 4, 128], bf16, tag="tr")
                        for j in range(4):
                            blk = grp * 4 + j
                            s, cb = blk // KT, blk % KT
                            nc.tensor.transpose(
                                ptr[:, j, :], x_nat[:, s, cb * 128:(cb + 1) * 128],
                                identb[:])
                        for j in range(4):
                            blk = grp * 4 + j
                            s, cb = blk // KT, blk % KT
                            nc.vector.tensor_copy(
                                xT[:, cb, s * 128:(s + 1) * 128], ptr[:, j, :])

                    # qk_nat (natural layout) + squares + norm partials
                    qk = rot.tile([128, SUB, 2 * C], bf16, tag="qk")
                    sq = rot1.tile([128, SUB, 2 * C], bf16, tag="sq")
                    for s in range(SUB):
                        for fc in range(3):
                            pqk = ps3.tile([128, FC_QK], f32, tag="qkmv")
                            for k in range(KT):
                                nc.tensor.matmul(
                                    pqk[:],
                                    xT[:, k, s * 128:(s + 1) * 128],
                                    wqk_t[fc][:, k, :],
                                    start=(k == 0), stop=(k == KT - 1))
                            nc.vector.tensor_copy(
                                qk[:, s, fc * FC_QK:(fc + 1) * FC_QK], pqk[:])
                            nc.scalar.activation(
                                sq[:, s, fc * FC_QK:(fc + 1) * FC_QK], pqk[:],
                                AF.Square)
                    # norm partials: [1, 512] = ones.T @ sq, accum over subtiles
                    for fc in range(3):
                        pmv = ps3.tile([1, FC_QK], f32, tag="qkmv")
                        for s in range(SUB):
                            nc.tensor.matmul(
                                pmv[:], ones_bf[:],
                                sq[:, s, fc * FC_QK:(fc + 1) * FC_QK],
                                start=(s == 0), stop=(s == SUB - 1))
                        nc.vector.tensor_add(
                            norm_acc[:, fc * FC_QK:(fc + 1) * FC_QK],
                            norm_acc[:, fc * FC_QK:(fc + 1) * FC_QK], pmv[:])

                    # Gram partial per head (bf16) -> accumulate in SBUF
                    for h in range(H):
                        pg = ps.tile([D, D], f32, tag="gr")
                        for s in range(SUB):
                            nc.tensor.matmul(
                                pg[:],
                                qk[:, s, h * D:(h + 1) * D],
                                qk[:, s, C + h * D:C + (h + 1) * D],
                                start=(s == 0), stop=(s == SUB - 1))
                        nc.vector.tensor_add(gram_sb[:, h, :], gram_sb[:, h, :],
                                             pg[:])

                    # vT per head (bf16) -> DRAM scratch
                    vtc = rot.tile([D, H, CHUNK], bf16, tag="vtc")
                    for h in range(H):
                        pvt = ps3.tile([D, CHUNK], f32, tag="vt")
                        for k in range(KT):
                            nc.tensor.matmul(
                                pvt[:],
                                wv_sb[:, k, h * D:(h + 1) * D],
                                xT[:, k, :],
                                start=(k == 0), stop=(k == KT - 1))
                        nc.scalar.copy(vtc[:, h, :], pvt[:])
                    nc.sync.dma_start(vt_dram[:, :, c0:c0 + CHUNK], vtc[:])

                # ================= Phase B (vectorized over heads) ===========
                rn = perb.tile([1, 2 * C], f32, tag="rn")
                nc.scalar.activation(rn[:], norm_acc[:], AF.Abs_reciprocal_sqrt,
                                     bias=eps_sb[:])
                nc.vector.tensor_tensor(
                    rn[0:1, 0:C].rearrange("o (h d) -> o h d", h=H),
                    rn[0:1, 0:C].rearrange("o (h d) -> o h d", h=H),
                    temp_sb[:].unsqueeze(2).to_broadcast([1, H, D]),
                    mybir.AluOpType.mult)

                # S = rq (x) rk packed like gram: bank0 heads 0-4, bank1 heads 5-7
                S_ps0 = ps3.tile([128, 480], f32, tag="qkmv", name="S_ps0")
                S_ps1 = ps3.tile([128, 288], f32, tag="qkmv", name="S_ps1")
                S_ps = [S_ps0, S_ps1]
                for h in range(H):
                    bank, off = (0, h * D) if h < 5 else (1, (h - 5) * D)
                    nc.tensor.matmul(
                        S_ps[bank][0:D, off:off + D],
                        rn[0:1, h * D:(h + 1) * D],
                        rn[0:1, C + h * D:C + (h + 1) * D],
                        start=True, stop=True)
                att = perb.tile([D, H, D], f32, tag="att")
                nc.vector.tensor_tensor(
                    att[:, 0:5, :].rearrange("p h d -> p (h d)"),
                    gram_sb[:, 0:5, :].rearrange("p h d -> p (h d)"),
                    S_ps[0][0:D, :], mybir.AluOpType.mult)
                nc.vector.tensor_tensor(
                    att[:, 5:8, :].rearrange("p h d -> p (h d)"),
                    gram_sb[:, 5:8, :].rearrange("p h d -> p (h d)"),
                    S_ps[1][0:D, :], mybir.AluOpType.mult)
                # softmax over last axis, all heads at once
                maxs = perb.tile([D, H], f32, tag="maxs")
                nc.vector.tensor_reduce(
                    maxs[:], att[:], axis=mybir.AxisListType.X,
                    op=mybir.AluOpType.max)
                nc.vector.tensor_tensor(
                    att[:], att[:],
                    maxs[:].unsqueeze(2).to_broadcast([D, H, D]),
                    mybir.AluOpType.subtract)
                atte = perb.tile([D, H, D], bf16, tag="atte")
                nc.scalar.activation(atte[:], att[:], AF.Exp)
                ssum = perb.tile([D, H], f32, tag="ssum")
                nc.vector.tensor_reduce(
                    ssum[:], atte[:], axis=mybir.AxisListType.X,
                    op=mybir.AluOpType.add)
                nc.vector.reciprocal(rsums[:], ssum[:])
                # attnT_h = atte_h.T (bf16), packed 4 per psum tile
                for grp in range(2):
                    pT = ps3.tile([D, 4, D], bf16, tag="vt")
                    for j in range(4):
                        h = grp * 4 + j
                        nc.tensor.transpose(
                            pT[:, j, :], atte[:, h, :], identb[0:D, 0:D])
                    nc.vector.tensor_copy(
                        attnT[:, grp * 4:(grp + 1) * 4, :], pT[:])

                # ================= Phase C =================
                for ci in range(NCH):
                    c0 = ci * CHUNK
                    vtc2 = rot.tile([D, H, CHUNK], bf16, tag="vtc2")
                    nc.sync.dma_start(vtc2[:], vt_dram[:, :, c0:c0 + CHUNK])
                    outh = rot.tile([D, H, CHUNK], bf16, tag="outh")
                    for h in range(H):
                        pav = ps3.tile([D, CHUNK], f32, tag="vt")
                        nc.tensor.matmul(pav[:], attnT[:, h, :], vtc2[:, h, :],
                                         start=True, stop=True)
                        nc.scalar.activation(outh[:, h, :], pav[:], AF.Copy,
                                             scale=rsums[:, h:h + 1])
                    for s in range(SUB):
                        y_sb = rot.tile([128, C], f32, tag="y_sb")
                        for fc in range(2):
                            pY = ps3.tile([128, FC_Y], f32, tag="qkmv")
                            for h in range(H):
                                nc.tensor.matmul(
                                    pY[:],
                                    outh[:, h, s * 128:(s + 1) * 128],
                                    wp_sb[:, h, fc * FC_Y:(fc + 1) * FC_Y],
                                    start=(h == 0), stop=(h == H - 1))
                            nc.vector.tensor_tensor(
                                y_sb[:, fc * FC_Y:(fc + 1) * FC_Y], pY[:],
                                b128[:, fc * FC_Y:(fc + 1) * FC_Y],
                                mybir.AluOpType.add)
                        nc.sync.dma_start(
                            y_d[b, c0 + s * 128:c0 + (s + 1) * 128, :], y_sb[:])

    nc.compile()
    _CACHE["nc"] = nc
    return nc


def _run(inputs, trace=False):
    from concourse.bass_utils import run_bass_kernel_spmd

    nc = _build()
    x = np.ascontiguousarray(np.asarray(inputs["x"], dtype=np.float32))
    shards = x.reshape(N_CORES, B_LOC, N, C)
    common = {
        "w_qkv": np.asarray(inputs["w_qkv"], dtype=np.float32),
        "temperature": np.asarray(inputs["temperature"], dtype=np.float32),
        "w_proj": np.asarray(inputs["w_proj"], dtype=np.float32),
        "b_proj": np.asarray(inputs["b_proj"], dtype=np.float32),
    }
    in_maps = [{"x": shards[i], **common} for i in range(N_CORES)]
    res = run_bass_kernel_spmd(nc, in_maps, list(range(N_CORES)), trace=trace)
    out = np.concatenate([res.results[i]["y"] for i in range(N_CORES)], axis=0)
    return out.reshape(B, N, C), res


def kernel(**inputs) -> np.ndarray:
    out, _ = _run(inputs, trace=False)
    return out


def kernel_traced(**inputs):
    """Returns (output, BassKernelResults with exec_time_ns). Requires NTFF hook."""
    out, res = _run(inputs, trace=True)
    return out, res
